# revision 1
# baseline (speedup 1.0000x reference)
"""Trainium2 Bass kernel for nn_DeformableAlignment (B=8, C=128, H=W=64).

Self-contained: accepts FULL inputs, shards one batch per NeuronCore
(8 cores, data-parallel over B), runs a Bass/Tile kernel, returns the
FULL output.

Per-core pipeline (one batch), processed in 8 column-bands of 512
targets so the 64 MiB similarity-map stream overlaps all compute:
  1. column sums of similarity_map via DMA-accumulate (SDMA CCE add)
     + PE ones-matmul partition reduction
  2. weighted_x = x * s  (s broadcast via K=1 PE matmul)
  3. 3x3 convs (27 out ch = 18 offset + 9 modulation) as PE matmuls
     over a zero-padded 66x66 layout; x-channels in bf16, weighted_x
     channels in fp32 (offset precision requires fp32 there)
  4. PE-transpose conv outputs into a target-major layout; DVE computes
     bilinear corner weights (mask and modulation folded in) and gather
     row indices; out-of-mask points get an out-of-bounds index
  5. indirect DMA gather of fp16 row-pairs from a transposed x copy in
     DRAM with bounds_check -> masked points move zero bytes (the data
     is ~99.9% masked)
  6. weighted sum over 9 taps x 4 corners with per-partition-scalar
     FMAs (scalar_tensor_tensor) in fp16
  7. store t-major [4096, 128] fp16; host transposes back
"""

import sys

for _p in ("/opt/trn_rl_repo",):
    if _p not in sys.path:
        sys.path.insert(0, _p)

import numpy as np
import ml_dtypes

import concourse.bass as bass
import concourse.tile as tile
from concourse import bacc, mybir
from concourse.bass import AP, IndirectOffsetOnAxis
from concourse.bass_utils import run_bass_kernel_spmd

ALU = mybir.AluOpType
ACTF = mybir.ActivationFunctionType
dt = mybir.dt

B, C, H, W, K = 8, 128, 64, 64, 9
HW = H * W                    # 4096
NBAND = 8
BAND = HW // NBAND            # 512 targets per band
NJ = BAND // 128              # 4 j-blocks per band
NK = NJ * K                   # 36 (j,k) pairs per band
PAD = 66
PADHW = PAD * PAD             # 4356
NIDX = NK                     # 36 gather indices per partition per band
XT_ROWS = 4224
PAD_ROW = 4200               # zeroed pad row for masked points

_CACHE = {}
KSKIP = set()


def _build_consts(b_off, b_mod):
    t = np.arange(HW)
    hh = (t // W).astype(np.float32)
    ww = (t % W).astype(np.float32)
    hhb = np.zeros((128, NBAND * NK), np.float32)
    wwb = np.zeros((128, NBAND * NK), np.float32)
    for band in range(NBAND):
        for j in range(NJ):
            tt = band * BAND + j * 128 + np.arange(128)
            for k in range(K):
                col = band * NK + j * K + k
                hhb[:, col] = hh[tt] + b_off[2 * k]
                wwb[:, col] = ww[tt] + b_off[2 * k + 1]
    bmod = np.tile(np.repeat(b_mod[None, :], 1, 0), (128, NJ)).astype(np.float32)
    ones_col = np.ones((128, 1), np.float32)
    ones_row = np.ones((1, 128), np.float32)
    ident = np.eye(128, dtype=np.float32)
    ident16 = np.eye(128, dtype=np.float16)
    return hhb, wwb, bmod, ones_col, ones_row, ident, ident16


def _conv_weights(w_off, w_mod):
    w_all = np.concatenate([w_off, w_mod], axis=0)  # [27, 256, 3, 3]
    lx = np.zeros((9, 128, 27), np.float32)
    lw = np.zeros((9, 128, 27), np.float32)
    for ty in range(3):
        for tx in range(3):
            tap = ty * 3 + tx
            lx[tap] = w_all[:, :128, ty, tx].T
            lw[tap] = w_all[:, 128:, ty, tx].T
    return np.ascontiguousarray(lx.astype(ml_dtypes.bfloat16)), np.ascontiguousarray(lw)


def build_kernel():
    nc = bacc.Bacc("TRN2", target_bir_lowering=False, debug=False,
                   num_devices=8)

    sim_d = nc.dram_tensor("sim", [HW, HW], dt.float32, kind="ExternalInput")
    x_d = nc.dram_tensor("x", [128, HW], dt.float32, kind="ExternalInput")
    wcx_d = nc.dram_tensor("wcx", [9, 128, 27], dt.bfloat16, kind="ExternalInput")
    wcw_d = nc.dram_tensor("wcw", [9, 128, 27], dt.float32, kind="ExternalInput")
    hhb_d = nc.dram_tensor("hhb", [128, NBAND * NK], dt.float32, kind="ExternalInput")
    wwb_d = nc.dram_tensor("wwb", [128, NBAND * NK], dt.float32, kind="ExternalInput")
    bmod_d = nc.dram_tensor("bmod", [128, NK], dt.float32, kind="ExternalInput")
    onesc_d = nc.dram_tensor("onesc", [128, 1], dt.float32, kind="ExternalInput")
    onesr_d = nc.dram_tensor("onesr", [1, 128], dt.float32, kind="ExternalInput")
    ident_d = nc.dram_tensor("ident", [128, 128], dt.float32, kind="ExternalInput")
    ident16_d = nc.dram_tensor("ident16", [128, 128], dt.float16, kind="ExternalInput")
    out_d = nc.dram_tensor("out_t", [128, HW], dt.float16, kind="ExternalOutput")
    xT_d = nc.dram_tensor("xT_scratch", [XT_ROWS, 128], dt.float16)
    xT2_d = nc.dram_tensor("xT2_scratch", [XT_ROWS, 256], dt.float16)
    wrap_d = nc.dram_tensor("idx_wrap_scratch", [2, 16 * (NIDX * 8)], dt.int16)
    import os as _os
    dbg = bool(_os.environ.get("KDBG"))
    global KSKIP
    KSKIP = set(_os.environ.get("KSKIP", "").split(","))
    dbg_d = None
    if dbg:
        dbg_d = {
            "dbg_s": nc.dram_tensor("dbg_s", [1, HW], dt.float32, kind="ExternalOutput"),
            "dbg_raw": nc.dram_tensor("dbg_raw", [128, NJ * 27], dt.float32, kind="ExternalOutput"),
            "dbg_idx": nc.dram_tensor("dbg_idx", [128, NIDX], dt.int16, kind="ExternalOutput"),
            "dbg_w00": nc.dram_tensor("dbg_w00", [128, NK], dt.float32, kind="ExternalOutput"),
            "dbg_g": nc.dram_tensor("dbg_g", [128, NIDX * 512], dt.float16, kind="ExternalOutput"),
            "dbg_xt": nc.dram_tensor("dbg_xt", [128, HW], dt.float16, kind="ExternalOutput"),
        }

    with tile.TileContext(nc) as tc:
        _emit(nc, tc, sim_d, x_d, wcx_d, wcw_d, hhb_d, wwb_d, bmod_d,
              onesc_d, onesr_d, ident_d, ident16_d, out_d, xT_d, xT2_d, wrap_d, dbg_d)
    nc.compile()
    return nc


def _emit(nc, tc, sim_d, x_d, wcx_d, wcw_d, hhb_d, wwb_d, bmod_d,
          onesc_d, onesr_d, ident_d, ident16_d, out_d, xT_d, xT2_d, wrap_d, dbg_d=None):
    from contextlib import ExitStack
    ctx = ExitStack()
    with ctx:
        consts = ctx.enter_context(tc.tile_pool(name="consts", bufs=1))
        statics = ctx.enter_context(tc.tile_pool(name="statics", bufs=1))
        accp = ctx.enter_context(tc.tile_pool(name="simacc", bufs=4))
        colp = ctx.enter_context(tc.tile_pool(name="colpart", bufs=1))
        smallp = ctx.enter_context(tc.tile_pool(name="small", bufs=2))
        mathp = ctx.enter_context(tc.tile_pool(name="math", bufs=2))
        outp = ctx.enter_context(tc.tile_pool(name="oacc", bufs=2))
        ps_conv = ctx.enter_context(tc.tile_pool(name="ps_conv", bufs=2, space="PSUM"))
        ps_s = ctx.enter_context(tc.tile_pool(name="ps_s", bufs=1, space="PSUM"))
        ps_t = ctx.enter_context(tc.tile_pool(name="ps_t", bufs=1, space="PSUM"))

        # ---- constants ---------------------------------------------------
        wcx = consts.tile([128, 9 * 27], dt.bfloat16, name="wcx_sb")
        nc.sync.dma_start(wcx[:], AP(wcx_d, 0, [[27, 128], [3456, 9], [1, 27]]))
        wcw = consts.tile([128, 9 * 27], dt.float32, name="wcw_sb")
        nc.sync.dma_start(wcw[:], AP(wcw_d, 0, [[27, 128], [3456, 9], [1, 27]]))

        hhb = consts.tile([128, NBAND * NK], dt.float32, name="hhb_sb")
        nc.sync.dma_start(hhb[:], hhb_d.ap())
        wwb = consts.tile([128, NBAND * NK], dt.float32, name="wwb_sb")
        nc.sync.dma_start(wwb[:], wwb_d.ap())
        bmod = consts.tile([128, NK], dt.float32, name="bmod_sb")
        nc.sync.dma_start(bmod[:], bmod_d.ap())
        onesc = consts.tile([128, 1], dt.float32, name="onesc_sb")
        nc.sync.dma_start(onesc[:], onesc_d.ap())
        onesr = consts.tile([1, 128], dt.float32, name="onesr_sb")
        nc.sync.dma_start(onesr[:], onesr_d.ap())
        ident = consts.tile([128, 128], dt.float32, name="ident_sb")
        nc.sync.dma_start(ident[:], ident_d.ap())
        ident16 = consts.tile([128, 128], dt.float16, name="ident16_sb")
        nc.sync.dma_start(ident16[:], ident16_d.ap())

        # ---- static working tensors --------------------------------------
        x_sb = statics.tile([128, HW], dt.float32, name="x_sb")
        nc.sync.dma_start(x_sb[:], x_d.ap())

        feat_x = statics.tile([128, PADHW], dt.bfloat16, name="feat_x")
        feat_w = statics.tile([128, PADHW], dt.float32, name="feat_w")
        nc.vector.memset(feat_x[:], 0.0)
        nc.vector.memset(feat_w[:], 0.0)
        fx3 = feat_x.rearrange("p (r c) -> p r c", c=PAD)
        fw3 = feat_w.rearrange("p (r c) -> p r c", c=PAD)
        nc.vector.tensor_copy(fx3[:, 1:65, 1:65], x_sb[:])

        # ---- transposed fp16 x into DRAM ---------------------------------
        x_f16 = statics.tile([128, HW], dt.float16, name="x_f16")
        nc.scalar.copy(x_f16[:], x_sb[:])
        xT_sb = statics.tile([128, HW], dt.float16, name="xT_sb")
        for jb in range(HW // 128):
            ptx = ps_t.tile([128, 128], dt.float16, name="ptx", tag="ptx")
            nc.tensor.transpose(ptx[:], x_f16[:, jb * 128:(jb + 1) * 128],
                                ident16[:])
            nc.scalar.copy(xT_sb[:, jb * 128:(jb + 1) * 128], ptx[:])
        nc.sync.dma_start(
            AP(xT_d, 0, [[128, 128], [128 * 128, HW // 128], [1, 128]]),
            xT_sb[:])
        if dbg_d:
            nc.sync.dma_start(dbg_d["dbg_xt"].ap(), xT_sb[:])
        zpad = statics.tile([128, 128], dt.float16, name="zpad")
        nc.vector.memset(zpad[:], 0.0)
        nc.sync.dma_start(
            AP(xT_d, HW * 128, [[128, 128], [1, 128]]), zpad[:])
        # xT2[r] = [xT[r], xT[r+64]] so one descriptor = all 4 corners
        nc.sync.dma_start(
            AP(xT2_d, 0, [[256 * 128, 33], [256, 128], [1, 128]]),
            AP(xT_d, 0, [[128 * 128, 33], [128, 128], [1, 128]]))
        nc.sync.dma_start(
            AP(xT2_d, 128, [[256 * 128, 32], [256, 128], [1, 128]]),
            AP(xT_d, 64 * 128, [[128 * 128, 32], [128, 128], [1, 128]]))
        nc.sync.dma_start(
            AP(xT2_d, 4096 * 256 + 128, [[256, 128], [1, 128]]), zpad[:])

        zpage = statics.tile([128, 128], dt.float16, name="zpage")
        nc.vector.memset(zpage[:], 0.0)

        # persistent, pre-zeroed gather destinations
        g_tiles = []
        for i in range(2):
            g = statics.tile([128, NIDX * 512], dt.float16, name=f"gbuf{i}")
            nc.vector.memset(g[:], 0.0)
            g_tiles.append(g)

        s_all = statics.tile([1, HW], dt.float32, name="s_all")
        out_c = statics.tile([128, HW], dt.float16, name="out_c")

        # ------------------------------------------------------------------
        SLAB = 2 * BAND   # 1024 columns per colsum slab

        slab_ps = {}

        def colsum(slab, rcc0=0, rcc1=16):
            if rcc0 == 0:
                slab_ps[slab] = [
                    ps_s.tile([1, BAND], dt.float32, name=f"ps_red{h}",
                              tag=f"ps_red{h}") for h in range(2)]
            ps_h = slab_ps[slab]
            for rcc in range(rcc0, rcc1):
                tl = accp.tile([128, 2 * SLAB], dt.float32, name="simtile",
                               tag="simtile")
                tl3 = tl.rearrange("p (c n) -> p c n", c=2)
                nc.gpsimd.dma_start(
                    tl3[:, :, :],
                    AP(sim_d, rcc * 2 * 128 * HW + slab * SLAB,
                       [[128 * HW, 2], [HW, 128], [1, SLAB]]))
                for c in range(2):
                    rc = rcc * 2 + c
                    for h in range(2):
                        nc.tensor.matmul(
                            ps_h[h][:], onesc[:],
                            tl3[:, c, h * BAND:(h + 1) * BAND],
                            start=(rc == 0), stop=(rc == 31))
            if rcc1 == 16:
                for h in range(2):
                    nc.scalar.copy(
                        s_all[:, slab * SLAB + h * BAND:
                              slab * SLAB + (h + 1) * BAND], ps_h[h][:])

        def weighted_x(band):
            pbc = ps_s.tile([128, BAND], dt.float32, name="ps_bc", tag="ps_bc")
            nc.tensor.matmul(pbc[:], onesr[:],
                             s_all[:, band * BAND:(band + 1) * BAND],
                             start=True, stop=True)
            nc.vector.tensor_tensor(
                fw3[:, 8 * band + 1:8 * band + 9, 1:65],
                x_sb[:, band * BAND:(band + 1) * BAND],
                pbc[:], ALU.mult)

        def band_compute(band, g_sb):
            # ---- conv ---------------------------------------------------
            pc = ps_conv.tile([27, BAND], dt.float32, name="pconv", tag="pconv")
            for ty in range(3):
                for tx in range(3):
                    tap = ty * 3 + tx
                    r0 = 8 * band + ty
                    rx = fx3[:, r0:r0 + 8, tx:tx + 64]
                    rw = fw3[:, r0:r0 + 8, tx:tx + 64]
                    nc.tensor.matmul(pc[:], wcx[:, tap * 27:(tap + 1) * 27],
                                     rx, start=(tap == 0), stop=False)
                    nc.tensor.matmul(pc[:], wcw[:, tap * 27:(tap + 1) * 27],
                                     rw, start=False, stop=(tap == 8))
            c27 = smallp.tile([27, BAND], dt.float32, name="c27", tag="c27")
            nc.scalar.copy(c27[:], pc[:])

            # ---- transpose to layout L [128, (j)(ch)] -------------------
            rawT = smallp.tile([128, NJ * 27], dt.float32, name="rawT",
                               tag="rawT")
            for j in range(NJ):
                ptr = ps_t.tile([128, 32], dt.float32, name="ptr", tag="ptr")
                nc.tensor.transpose(ptr[:, :27],
                                    c27[:, j * 128:(j + 1) * 128],
                                    ident[:27, :27])
                nc.scalar.copy(rawT[:, j * 27:(j + 1) * 27], ptr[:, :27])

            r3 = rawT.rearrange("p (j c) -> p j c", c=27)
            offh_v = r3[:, :, 0:17:2]    # [128, 4, 9]
            offw_v = r3[:, :, 1:18:2]
            mod_v = r3[:, :, 18:27]

            def mt(nm, dtype=dt.float32):
                return mathp.tile([128, NK], dtype, name=nm, tag=nm)

            cs = band * NK
            off_h = mt("off_h"); off_w = mt("off_w")
            nc.vector.tensor_tensor(off_h[:], offh_v, hhb[:, cs:cs + NK], ALU.add)
            nc.vector.tensor_tensor(off_w[:], offw_v, wwb[:, cs:cs + NK], ALU.add)

            # floor via int roundtrip + rounding-mode-agnostic fixup
            fih = mt("fih", dt.int32); fiw = mt("fiw", dt.int32)
            f_h = mt("f_h"); f_w = mt("f_w")
            ch = mt("chf"); cw = mt("cwf")
            nc.vector.tensor_copy(fih[:], off_h[:])
            nc.vector.tensor_copy(f_h[:], fih[:])
            nc.vector.tensor_tensor(ch[:], f_h[:], off_h[:], ALU.is_gt)
            nc.vector.tensor_tensor(f_h[:], f_h[:], ch[:], ALU.subtract)
            nc.vector.tensor_copy(fiw[:], off_w[:])
            nc.vector.tensor_copy(f_w[:], fiw[:])
            nc.vector.tensor_tensor(cw[:], f_w[:], off_w[:], ALU.is_gt)
            nc.vector.tensor_tensor(f_w[:], f_w[:], cw[:], ALU.subtract)

            lh = mt("lh"); lw = mt("lw")
            nc.vector.tensor_tensor(lh[:], off_h[:], f_h[:], ALU.subtract)
            nc.vector.tensor_tensor(lw[:], off_w[:], f_w[:], ALU.subtract)

            # mask = (0<=off_h<=63) & (0<=off_w<=63)
            mh = mt("mh"); mw = mt("mw"); mask = mt("mask")
            nc.vector.tensor_scalar(mh[:], off_h[:], 0.0, None, ALU.is_ge)
            nc.vector.scalar_tensor_tensor(mh[:], off_h[:], 63.0, mh[:],
                                           ALU.is_le, ALU.mult)
            nc.vector.tensor_scalar(mw[:], off_w[:], 0.0, None, ALU.is_ge)
            nc.vector.scalar_tensor_tensor(mw[:], off_w[:], 63.0, mw[:],
                                           ALU.is_le, ALU.mult)
            nc.vector.tensor_tensor(mask[:], mh[:], mw[:], ALU.mult)

            # modulation * mask
            smod = mt("smod"); mm = mt("mmw")
            nc.vector.tensor_tensor(smod[:], mod_v, bmod[:], ALU.add)
            nc.scalar.activation(smod[:], smod[:], ACTF.Sigmoid)
            nc.vector.tensor_tensor(mm[:], smod[:], mask[:], ALU.mult)

            # corner weights
            t1 = mt("t1"); a0 = mt("a0"); t2 = mt("t2"); t3 = mt("t3")
            w00 = mt("w00"); w10 = mt("w10")
            nc.vector.tensor_tensor(t1[:], lh[:], mm[:], ALU.mult)
            nc.vector.tensor_tensor(a0[:], mm[:], t1[:], ALU.subtract)
            nc.vector.tensor_tensor(t2[:], lw[:], a0[:], ALU.mult)
            nc.vector.tensor_tensor(w00[:], a0[:], t2[:], ALU.subtract)
            w01 = t2
            nc.vector.tensor_tensor(t3[:], lw[:], t1[:], ALU.mult)
            nc.vector.tensor_tensor(w10[:], t1[:], t3[:], ALU.subtract)
            w11 = t3

            # gather indices: one quad-row per (j,k); masked points read
            # the zeroed pad row PAD_ROW
            i0f = mt("i0f"); i0m = mt("i0m")
            nc.vector.scalar_tensor_tensor(i0f[:], f_h[:], 64.0, f_w[:],
                                           ALU.mult, ALU.add)
            nc.vector.tensor_scalar(i0f[:], i0f[:], float(PAD_ROW), None,
                                    ALU.subtract)
            nc.vector.tensor_tensor(i0m[:], i0f[:], mask[:], ALU.mult)
            idx = mathp.tile([128, NIDX], dt.int16, name="idx", tag="idx")
            nc.vector.tensor_scalar(idx[:], i0m[:], float(PAD_ROW), None,
                                    ALU.add)

            if "wrap" in KSKIP:
                return
            # ---- wrapped index layout for dma_gather --------------------
            # descriptor i = jp*128 + p lives at [i%16, i//16]; build via
            # DRAM roundtrip, replicated to all 8 Q7 core groups
            wslot = band % 2
            wr = AP(wrap_d, wslot * 16 * NIDX * 8,
                    [[1, 8], [NIDX * 8, 16], [8, NIDX]])
            nc.sync.dma_start(wr, idx[:])
            idxw = mathp.tile([128, NIDX * 8], dt.int16, name="idxw", tag="idxw")
            nc.sync.dma_start(
                idxw[:],
                AP(wrap_d, wslot * 16 * NIDX * 8,
                   [[0, 8], [NIDX * 8, 16], [1, NIDX * 8]]))

            if "gather" in KSKIP:
                return
            # ---- gather (masked points read zeros from the pad row) -----
            # single_packet packetization only works <=1024 idxs per call
            g3 = g_sb.rearrange("p (n e) -> p n e", e=512)
            CH = 1024
            pos = 0
            while pos < NIDX * 128:
                n = min(CH, NIDX * 128 - pos)
                nc.gpsimd.dma_gather(
                    g3[:, pos // 128:(pos + n) // 128, :],
                    AP(xT2_d, 0, [[256, XT_ROWS - 2], [1, 512]]),
                    idxw[:, pos // 16:(pos + n) // 16],
                    n,
                    n,
                    512,
                    elem_step=256,
                    single_packet=True,
                )
                pos += n

            if "stt" in KSKIP:
                return
            # ---- weighted accumulate ------------------------------------
            acc = outp.tile([128, BAND], dt.float16, name="oacc", tag="oacc")
            for j in range(NJ):
                aj = acc[:, j * 128:(j + 1) * 128]
                for k in range(K):
                    col = j * K + k
                    base = col * 512
                    ga0 = g_sb[:, base:base + 128]
                    gb0 = g_sb[:, base + 128:base + 256]
                    ga1 = g_sb[:, base + 256:base + 384]
                    gb1 = g_sb[:, base + 384:base + 512]
                    sc = lambda wv: wv[:, col:col + 1]
                    nc.vector.scalar_tensor_tensor(
                        aj, ga0, sc(w00), zpage[:] if k == 0 else aj,
                        ALU.mult, ALU.add)
                    nc.vector.scalar_tensor_tensor(aj, ga1, sc(w01), aj,
                                                   ALU.mult, ALU.add)
                    nc.vector.scalar_tensor_tensor(aj, gb0, sc(w10), aj,
                                                   ALU.mult, ALU.add)
                    nc.vector.scalar_tensor_tensor(aj, gb1, sc(w11), aj,
                                                   ALU.mult, ALU.add)

            if dbg_d and band == 0:
                nc.sync.dma_start(dbg_d["dbg_raw"].ap(), rawT[:])
                nc.sync.dma_start(dbg_d["dbg_idx"].ap(), idx[:])
                nc.sync.dma_start(dbg_d["dbg_w00"].ap(), w00[:])
                nc.sync.dma_start(dbg_d["dbg_g"].ap(), g_sb[:])

            # ---- transpose to [c, t] and stage in SBUF ------------------
            for j in range(NJ):
                pso = ps_t.tile([128, 128], dt.float16, name="pso", tag="ptx")
                nc.tensor.transpose(pso[:], acc[:, j * 128:(j + 1) * 128],
                                    ident16[:])
                nc.scalar.copy(
                    out_c[:, band * BAND + j * 128:band * BAND + (j + 1) * 128],
                    pso[:])

        def store_out():
            nc.sync.dma_start(out_d.ap(), out_c[:])

        def dump_s():
            if dbg_d:
                nc.sync.dma_start(dbg_d["dbg_s"].ap(), s_all[:])

        # ---- pipelined emission ------------------------------------------
        # slabs 0-1 up front (band 0 needs both); slabs 2-3 dribbled in
        # between early band computes so they never block the pipeline
        colsum(0)
        colsum(1)
        weighted_x(0)
        weighted_x(1)
        weighted_x(2)
        for band in range(NBAND):
            if band % 2 == 0 and band // 2 + 2 < NBAND // 2:
                colsum(band // 2 + 2)
            if band + 3 < NBAND:
                weighted_x(band + 3)
            band_compute(band, g_tiles[band % 2])
        store_out()
        dump_s()


def kernel(**inputs):
    x = np.asarray(inputs["x"], np.float32)
    sim = np.asarray(inputs["similarity_map"], np.float32)
    w_off = np.asarray(inputs["w_off"], np.float32)
    b_off = np.asarray(inputs["b_off"], np.float32)
    w_mod = np.asarray(inputs["w_mod"], np.float32)
    b_mod = np.asarray(inputs["b_mod"], np.float32)

    if "nc" not in _CACHE:
        _CACHE["nc"] = build_kernel()
    nc = _CACHE["nc"]

    hhb, wwb, bmod, onesc, onesr, ident, ident16 = _build_consts(b_off, b_mod)
    wcx, wcw = _conv_weights(w_off, w_mod)

    in_maps = []
    for b in range(B):
        in_maps.append({
            "sim": np.ascontiguousarray(sim[b]),
            "x": np.ascontiguousarray(x[b].reshape(C, HW)),
            "wcx": wcx, "wcw": wcw,
            "hhb": hhb, "wwb": wwb, "bmod": bmod,
            "onesc": onesc, "onesr": onesr,
            "ident": ident, "ident16": ident16,
        })

    res = run_bass_kernel_spmd(nc, in_maps, core_ids=list(range(B)))
    _CACHE["last_res"] = res
    outs = []
    for b in range(B):
        ot = res.results[b]["out_t"]
        outs.append(ot.astype(np.float32).reshape(C, H, W))
    return np.stack(outs)

